# revision 1
# baseline (speedup 1.0000x reference)
"""Trainium2 Bass kernel for nn_Attention_31147102831130.

Math (per token): qkv = x@Wqkv+b; per-position attention over the HEADS axis:
  q,k,v: [H=16, Dh=64]; A = softmax(q k^T / sqrt(1024)); o = A v (flat 1024)
  y = o@Wo + bo.

Sharding: pure data-parallel over batch: 8 cores x 2 batches (2048 tokens).

Per-core pipeline (token-major, 16 tiles of 128 tokens):
  PE    : QKV projection (bf16), o transposes, output projection (bf16)
  DVE   : bias adds, score products q_h*k_t + d-tree-reduce, A*V + t-tree-reduce
  ACT   : exp
All weights SBUF-resident. Host pre-permutes Wqkv columns to [Q|K|V] head-major
and pre-transposes x so no on-device transposes are needed for phase 1.
"""

import numpy as np
import ml_dtypes

B, S, C = 16, 1024, 1024
E, H, DH = 1024, 16, 64
NCORES = 8
TOK = B * S // NCORES      # 2048 tokens per core
PT = 128                   # tokens per tile
NT = TOK // PT             # 16 tiles
KC = C // 128              # 8 contraction chunks

_CACHE = {}
import os
_GPMODE = int(os.environ.get("KERNEL_GPSIMD", "0"))
_DENSE = os.environ.get("KERNEL_DENSE", "0") == "1"
_GPTILE = int(os.environ.get("KERNEL_GPTILE", "0"))  # every Nth tile attn on gpsimd (0=off)
_EDIRECT = os.environ.get("KERNEL_EDIRECT", "0") == "1"  # unnormalized E in AV; fold 1/Z into o extraction
_SCAN = os.environ.get("KERNEL_SCAN", "0") == "1"  # segmented-sum via tensor_tensor_scan instead of tree adds
_PIPE = os.environ.get("KERNEL_PIPE", "0") == "1"  # software-pipeline: emit phase1(i+1) before attention(i)
_NH = int(os.environ.get("KERNEL_NH", "1"))  # number of h-groups for products/trees
_BUFS_QKV = int(os.environ.get("KB_QKV", "2"))
_BUFS_ATTN = int(os.environ.get("KB_ATTN", "2"))
_BUFS_PROD = int(os.environ.get("KB_PROD", "2"))
_BUFS_O = int(os.environ.get("KB_O", "2"))
_BUFS_Y = int(os.environ.get("KB_Y", "2"))
_BUFS_PSQKV = int(os.environ.get("KB_PSQKV", "2"))


def _build_nc():
    import os
    import concourse.bass as bass
    import concourse.mybir as mybir
    from concourse.tile import TileContext
    from concourse.masks import make_identity
    from concourse.bass import ts, ds

    bf16 = mybir.dt.bfloat16
    f32 = mybir.dt.float32
    AF = mybir.ActivationFunctionType
    OP = mybir.AluOpType

    nc = bass.Bass()

    xT_d = nc.declare_dram_parameter("xT", [KC, 128, TOK], bf16, isOutput=False)
    wqkv_d = nc.declare_dram_parameter("wqkv", [KC, 128, 3 * E], bf16, isOutput=False)
    bqkv_d = nc.declare_dram_parameter("bqkv", [1, 3 * E], bf16, isOutput=False)
    wo_d = nc.declare_dram_parameter("wo", [KC, 128, E], bf16, isOutput=False)
    bo_d = nc.declare_dram_parameter("bo", [1, E], bf16, isOutput=False)
    y_d = nc.declare_dram_parameter("y", [TOK, E], f32, isOutput=True)

    with TileContext(nc) as tc:
        with (
            tc.tile_pool(name="wpool", bufs=1) as wp,
            tc.tile_pool(name="qkvpool", bufs=_BUFS_QKV) as qp,
            tc.tile_pool(name="attnpool", bufs=_BUFS_ATTN) as ap_,
            tc.tile_pool(name="opool", bufs=_BUFS_O) as op_,
            tc.tile_pool(name="prodpool", bufs=_BUFS_PROD) as pp,
            tc.tile_pool(name="ypool", bufs=_BUFS_Y) as yp,
            tc.tile_pool(name="psqkv", bufs=_BUFS_PSQKV, space="PSUM") as ps_qkv,
            tc.tile_pool(name="pst", bufs=int(os.environ.get("KB_PST", "2")), space="PSUM") as ps_t,
            tc.tile_pool(name="psy", bufs=int(os.environ.get("KB_PSY", "2")), space="PSUM") as ps_y,
        ):
            # ---- persistent weights ----
            xall = wp.tile([128, KC, TOK], bf16)
            _XSPL = int(os.environ.get("KERNEL_XSPL", "8"))
            if os.environ.get("KERNEL_XFIRST", "0") == "1":
                for sl in range(_XSPL):
                    nc.sync.dma_start(
                        xall[:, :, ds(sl * (TOK // _XSPL), TOK // _XSPL)],
                        xT_d[:, :, ds(sl * (TOK // _XSPL), TOK // _XSPL)].rearrange(
                            "k p t -> p k t"
                        ),
                    )
            wqkv_sb = wp.tile([128, KC, 3 * E], bf16)
            _WSPL = int(os.environ.get("KERNEL_WSPL", "12"))
            for sl in range(_WSPL):
                w_ = 3 * E // _WSPL
                nc.sync.dma_start(
                    wqkv_sb[:, :, ds(sl * w_, w_)],
                    wqkv_d[:, :, ds(sl * w_, w_)].rearrange("k p e -> p k e"),
                )
            wo_sb = wp.tile([128, KC, E], bf16)
            _OSPL = int(os.environ.get("KERNEL_OSPL", "2"))
            for sl in range(_OSPL):
                w2 = E // _OSPL
                nc.sync.dma_start(
                    wo_sb[:, :, ds(sl * w2, w2)],
                    wo_d[:, :, ds(sl * w2, w2)].rearrange("k p e -> p k e"),
                )
            bq1 = wp.tile([1, 3 * E], bf16)
            nc.sync.dma_start(bq1, bqkv_d[:, :])
            bo1 = wp.tile([1, E], bf16)
            nc.sync.dma_start(bo1, bo_d[:, :])
            ones = wp.tile([1, 128], bf16)
            nc.vector.memset(ones, 1.0)
            ident = wp.tile([128, 128], bf16)
            make_identity(nc, ident)
            mask_d = wp.tile([128, DH], bf16)   # 0,1,1,...  resets scan at d=0
            nc.vector.memset(mask_d, 1.0)
            nc.vector.memset(mask_d[:, 0:1], 0.0)
            mask_t = wp.tile([128, H], bf16)
            nc.vector.memset(mask_t, 1.0)
            nc.vector.memset(mask_t[:, 0:1], 0.0)
            if os.environ.get("KERNEL_XFIRST", "0") != "1":
                for sl in range(_XSPL):
                    nc.sync.dma_start(
                        xall[:, :, ds(sl * (TOK // _XSPL), TOK // _XSPL)],
                        xT_d[:, :, ds(sl * (TOK // _XSPL), TOK // _XSPL)].rearrange(
                            "k p t -> p k t"
                        ),
                    )

            def phase1(i):
                # ---- phase 1: qkv = x @ Wqkv + b   (token-major [tok, 3E])
                qkv = qp.tile([128, 3 * E], bf16)
                for j in range(6):  # 6 x 512 output channels
                    ps = ps_qkv.tile([128, 512], f32)
                    for k in range(KC):
                        nc.tensor.matmul(
                            ps,
                            xall[:, k, ts(i, PT)],
                            wqkv_sb[:, k, ds(j * 512, 512)],
                            start=(k == 0),
                            stop=False,
                        )
                    nc.tensor.matmul(
                        ps,
                        ones[0:1, :],
                        bq1[0:1, ds(j * 512, 512)],
                        start=False,
                        stop=True,
                    )
                    nc.scalar.copy(qkv[:, ds(j * 512, 512)], ps)
                return qkv

            def attention(i, qkv):
                gp_tile = _GPTILE > 0 and (i % _GPTILE) == (_GPTILE - 1)
                HH = H // _NH
                k_ap = qkv[:, E : 2 * E].rearrange("p (u t d) -> p u t d", t=H, u=1)
                v_ap = qkv[:, 2 * E : 3 * E].rearrange("p (t d) -> p d t", t=H)

                # ---- phase 2a: scores product + tree reduce over d (h-halves)
                ex = ap_.tile([128, H, H], bf16 if _EDIRECT else f32)
                for g in range(_NH):
                    q_ap = qkv[:, ds(g * HH * DH, HH * DH)].rearrange(
                        "p (h u d) -> p h u d", h=HH, u=1
                    )
                    psc = pp.tile([128, HH, H, DH], bf16, tag="prod")
                    if _DENSE:
                        kd = qkv[:, E : 2 * E].rearrange(
                            "p (u t d) -> p u t d", t=H, u=1
                        )
                        for hh in range(HH):
                            nc.vector.tensor_mul(
                                psc[:, hh : hh + 1, :, :],
                                q_ap[:, hh : hh + 1, :, :].broadcast_to(
                                    [128, 1, H, DH]
                                ),
                                kd.broadcast_to([128, 1, H, DH]),
                            )
                    else:
                        eng_p = nc.gpsimd if gp_tile else nc.vector
                        eng_p.tensor_mul(
                            psc,
                            q_ap.broadcast_to([128, HH, H, DH]),
                            k_ap.broadcast_to([128, HH, H, DH]),
                        )
                    if _SCAN:
                        nc.vector.tensor_tensor_scan(
                            psc.rearrange("p h t d -> p (h t) d"),
                            mask_d.rearrange("p (u d) -> p u d", u=1).broadcast_to(
                                [128, HH * H, DH]
                            ),
                            psc.rearrange("p h t d -> p (h t) d"),
                            0.0,
                            OP.mult,
                            OP.add,
                        )
                        s_src = psc[:, :, :, DH - 1]
                    else:
                        eng_s = nc.gpsimd if (gp_tile or (g == 1 and _GPMODE in (1, 2))) else nc.vector
                        w = DH
                        while w > 1:
                            w //= 2
                            eng_s.tensor_add(
                                psc[:, :, :, 0:w],
                                psc[:, :, :, 0:w],
                                psc[:, :, :, w : 2 * w],
                            )
                        s_src = psc[:, :, :, 0]
                    # softmax exp (no max-sub; |scores/32| is small)
                    nc.scalar.activation(
                        ex[:, ds(g * HH, HH), :],
                        s_src,
                        AF.Exp,
                        scale=float(E) ** -0.5,
                    )
                zr = ap_.tile([128, H], f32)
                nc.vector.reduce_sum(zr, ex, axis=mybir.AxisListType.X)
                nc.vector.reciprocal(zr, zr)
                if not _EDIRECT:
                    a_bf = ap_.tile([128, H, H], bf16)
                    nc.vector.tensor_mul(
                        a_bf,
                        ex,
                        zr.rearrange("p (h u) -> p h u", u=1).broadcast_to([128, H, H]),
                    )
                else:
                    a_bf = ex

                # ---- phase 2b: o = A @ v per token: product + tree over t
                vt = ap_.tile([128, DH, H], bf16)  # v as [d, t]
                nc.scalar.copy(vt, v_ap)
                o_c = op_.tile([128, E], bf16)  # [tok, (h d)]
                vt_b = vt.rearrange("p (u d) t -> p u d t", u=1)
                for g in range(_NH):
                    pav = pp.tile([128, HH, DH, H], bf16, tag="prod")
                    (nc.gpsimd if gp_tile else nc.vector).tensor_mul(
                        pav,
                        a_bf[:, ds(g * HH, HH), :]
                        .rearrange("p h (u t) -> p h u t", u=1)
                        .broadcast_to([128, HH, DH, H]),
                        vt_b.broadcast_to([128, HH, DH, H]),
                    )
                    if _SCAN:
                        nc.vector.tensor_tensor_scan(
                            pav.rearrange("p h d t -> p (h d) t"),
                            mask_t.rearrange("p (u t) -> p u t", u=1).broadcast_to(
                                [128, HH * DH, H]
                            ),
                            pav.rearrange("p h d t -> p (h d) t"),
                            0.0,
                            OP.mult,
                            OP.add,
                        )
                    else:
                        eng_a = nc.gpsimd if (gp_tile or (g == 1 and _GPMODE in (1, 3))) else nc.vector
                        w = H
                        while w > 1:
                            w //= 2
                            eng_a.tensor_add(
                                pav[:, :, :, 0:w],
                                pav[:, :, :, 0:w],
                                pav[:, :, :, w : 2 * w],
                            )
                    if _EDIRECT:
                        for hh in range(HH):
                            nc.scalar.activation(
                                o_c[:, ds((g * HH + hh) * DH, DH)],
                                pav[:, hh, :, 0],
                                AF.Copy,
                                scale=zr[:, g * HH + hh : g * HH + hh + 1],
                            )
                    else:
                        nc.scalar.copy(
                            o_c[:, ds(g * HH * DH, HH * DH)].rearrange(
                                "p (h d) -> p h d", h=HH
                            ),
                            pav[:, :, :, H - 1] if _SCAN else pav[:, :, :, 0],
                        )

                # ---- transpose o to channel-major for o_proj
                pst = ps_t.tile([128, KC, 128], bf16)
                for m in range(KC):
                    nc.tensor.transpose(pst[:, m, :], o_c[:, ts(m, 128)], ident)
                oT = op_.tile([128, KC, 128], bf16)
                nc.scalar.copy(oT[:, 0:4, :], pst[:, 0:4, :])
                nc.scalar.copy(oT[:, 4:8, :], pst[:, 4:8, :])

                # ---- phase 3: y = o @ Wo + bo
                ysb = yp.tile([128, E], f32)
                for j in range(2):
                    psy = ps_y.tile([128, 512], f32)
                    for m in range(KC):
                        nc.tensor.matmul(
                            psy,
                            oT[:, m, :],
                            wo_sb[:, m, ds(j * 512, 512)],
                            start=(m == 0),
                            stop=False,
                        )
                    nc.tensor.matmul(
                        psy,
                        ones[0:1, :],
                        bo1[0:1, ds(j * 512, 512)],
                        start=False,
                        stop=True,
                    )
                    nc.scalar.copy(ysb[:, ds(j * 512, 512)], psy)
                nc.sync.dma_start(y_d[ts(i, PT), :], ysb)

            if _PIPE:
                qkv_next = phase1(0)
                for i in range(NT):
                    qkv_cur = qkv_next
                    if i + 1 < NT:
                        qkv_next = phase1(i + 1)
                    attention(i, qkv_cur)
            else:
                for i in range(NT):
                    attention(i, phase1(i))

    _legalize_waits(nc, mybir)
    return nc


def _legalize_waits(nc, mybir):
    """This walrus build allows only ONE sync wait per engine instruction.
    Split extra waits into standalone same-engine EventSemaphore insts."""
    for f in nc.m.functions:
        for b in f.blocks:
            newl = []
            for inst in b.instructions:
                si = getattr(inst, "sync_info", None)
                ow = list(si.on_wait) if si and si.on_wait else []
                if len(ow) > 1:
                    for w in ow[:-1]:
                        newl.append(
                            mybir.InstEventSemaphore(
                                name=f"WS-{nc.next_id()}",
                                engine=inst.engine,
                                sync_info=mybir.SyncInfo(on_wait=[w], on_update=[]),
                            )
                        )
                    si.on_wait = [ow[-1]]
                newl.append(inst)
            b.instructions = newl


def _prep_weights(w_qkv, b_qkv, w_o, b_o):
    # permute fused-qkv columns: orig e = h*192 + part*64 + d
    #                           new  e = part*1024 + h*64 + d
    part, h, d = np.meshgrid(
        np.arange(3), np.arange(H), np.arange(DH), indexing="ij"
    )
    perm = (h * 192 + part * 64 + d).reshape(-1)
    wq = np.ascontiguousarray(w_qkv[:, perm]).astype(ml_dtypes.bfloat16)
    bq = np.ascontiguousarray(b_qkv[perm]).astype(ml_dtypes.bfloat16)[None, :]
    wo = np.ascontiguousarray(w_o).astype(ml_dtypes.bfloat16)
    return (
        wq.reshape(KC, 128, 3 * E),
        bq,
        wo.reshape(KC, 128, E),
        np.asarray(b_o).astype(ml_dtypes.bfloat16)[None, :],
    )


def kernel(x, w_qkv, b_qkv, w_o, b_o):
    from concourse.bass_utils import run_bass_kernel_spmd

    if "nc" not in _CACHE:
        _CACHE["nc"] = _build_nc()
    nc = _CACHE["nc"]

    wq, bq, wo, bo = _prep_weights(
        np.asarray(w_qkv, np.float32),
        np.asarray(b_qkv, np.float32),
        np.asarray(w_o, np.float32),
        np.asarray(b_o, np.float32),
    )
    x = np.asarray(x, np.float32)
    in_maps = []
    for c in range(NCORES):
        xc = x[2 * c : 2 * c + 2].reshape(TOK, C)
        xT = np.ascontiguousarray(xc.T).astype(ml_dtypes.bfloat16)
        in_maps.append(
            {
                "xT": xT.reshape(KC, 128, TOK),
                "wqkv": wq,
                "bqkv": bq,
                "wo": wo,
                "bo": bo,
            }
        )

    res = run_bass_kernel_spmd(nc, in_maps, core_ids=list(range(NCORES)))
    out = np.empty((B, S, E), np.float32)
    for c in range(NCORES):
        out[2 * c : 2 * c + 2] = res.results[c]["y"].reshape(2, S, E)
    return out



# revision 37
# speedup vs baseline: 1.1278x; 1.1278x over previous
"""Trainium2 Bass kernel for nn_Attention_31147102831130.

Math (per token): qkv = x@Wqkv+b; per-position attention over the HEADS axis:
  q,k,v: [H=16, Dh=64]; A = softmax(q k^T / sqrt(1024)); o = A v (flat 1024)
  y = o@Wo + bo.

Sharding: pure data-parallel over batch: 8 cores x 2 batches (2048 tokens).

Per-core pipeline (token-major, 16 tiles of 128 tokens), two decoupled
attention lanes split by query-head (the softmax over t is per-h row, so
head ranges are fully independent):
  PE    : QKV projection (bf16), o transposes, output projection (bf16)
  DVE   : lane A = heads [0, HL): score products + d-tree, AV products + t-tree
  GPSIMD: lane B = heads [HL, 16): same (cost-model rate 1.98 ns/elem vs 0.52)
  ACT   : PSUM->SBUF copies, per-lane exp, per-head 1/Z-scaled o extraction
1/Z is folded into the o extraction (ACT scale), so AV runs on unnormalized
exp(S) and neither lane's AV waits on the other engine.
All weights SBUF-resident. Host pre-permutes Wqkv columns to [Q|K|V] head-major
and pre-transposes x so no on-device transposes are needed for phase 1.
"""

import numpy as np
import ml_dtypes
import os

B, S, C = 16, 1024, 1024
E, H, DH = 1024, 16, 64
NCORES = 8
TOK = B * S // NCORES      # 2048 tokens per core
PT = 128                   # tokens per tile
NT = TOK // PT             # 16 tiles
KC = C // 128              # 8 contraction chunks

_CACHE = {}
# Lane boundary: DVE owns softmax rows (heads) [0, HL); GPSIMD owns [HL, 16).
_HL = int(os.environ.get("K_HL", "13"))
# Products may be split at a different head index for balance (the tree/exp
# owner is the lane; off-lane products just add one cross-engine dep).
_SP = int(os.environ.get("K_SP", "11"))   # score products: DVE heads [0, SP)
_AP = int(os.environ.get("K_AP", "13"))   # AV products: DVE heads [0, AP)
_PIPE = os.environ.get("K_PIPE", "1") == "1"


def _build_nc():
    import concourse.bass as bass
    import concourse.mybir as mybir
    from concourse.tile import TileContext
    from concourse.masks import make_identity
    from concourse.bass import ts, ds

    bf16 = mybir.dt.bfloat16
    f32 = mybir.dt.float32
    AF = mybir.ActivationFunctionType

    nc = bass.Bass()

    xT_d = nc.declare_dram_parameter("xT", [KC, 128, TOK], bf16, isOutput=False)
    wqkv_d = nc.declare_dram_parameter("wqkv", [KC, 128, 3 * E], bf16, isOutput=False)
    bqkv_d = nc.declare_dram_parameter("bqkv", [1, 3 * E], bf16, isOutput=False)
    wo_d = nc.declare_dram_parameter("wo", [KC, 128, E], bf16, isOutput=False)
    bo_d = nc.declare_dram_parameter("bo", [1, E], bf16, isOutput=False)
    y_d = nc.declare_dram_parameter("y", [TOK, E], f32, isOutput=True)

    _W1G = os.environ.get("K_W1G", "0") == "1"  # w=1 level (1x mode) on gpsimd

    def tree(eng, p, axlen, eng_last=None, stop_w=1):
        # in-place halving tree over the innermost axis of p [128, a, b, w]
        w = axlen
        while w > stop_w:
            w //= 2
            e = eng_last if (w == 1 and eng_last is not None) else eng
            e.tensor_add(p[:, :, :, 0:w], p[:, :, :, 0:w], p[:, :, :, w : 2 * w])

    with TileContext(nc) as tc:
        with (
            tc.tile_pool(name="wpool", bufs=1) as wp,
            tc.tile_pool(name="qkvpool", bufs=int(os.environ.get("KB_QKV", "2"))) as qp,
            tc.tile_pool(name="attnpool", bufs=int(os.environ.get("KB_ATTN", "2"))) as ap_,
            tc.tile_pool(name="opool", bufs=int(os.environ.get("KB_O", "2"))) as op_,
            tc.tile_pool(name="prodpool", bufs=int(os.environ.get("KB_PROD", "2"))) as pp,
            tc.tile_pool(name="ypool", bufs=int(os.environ.get("KB_Y", "2"))) as yp,
            tc.tile_pool(name="psqkv", bufs=int(os.environ.get("KB_PSQKV", "2")), space="PSUM") as ps_qkv,
            tc.tile_pool(name="pst", bufs=int(os.environ.get("KB_PST", "2")), space="PSUM") as ps_t,
            tc.tile_pool(name="psy", bufs=int(os.environ.get("KB_PSY", "2")), space="PSUM") as ps_y,
        ):
            # ---- persistent weights ----
            # Order DMAs so tile 0 can start ASAP: biases, then Wqkv q+k
            # column chunks, then x slice 0, then Wqkv v chunks, rest of x, Wo.
            bq1 = wp.tile([1, 3 * E], bf16)
            nc.sync.dma_start(bq1, bqkv_d[:, :])
            bo1 = wp.tile([1, E], bf16)
            nc.sync.dma_start(bo1, bo_d[:, :])
            wqkv_sb = wp.tile([128, KC, 3 * E], bf16)
            _WSPL = int(os.environ.get("KERNEL_WSPL", "12"))
            w_ = 3 * E // _WSPL
            nqk = (2 * E) // w_  # slices covering q+k columns
            xall = wp.tile([128, KC, TOK], bf16)
            _XSPL = int(os.environ.get("KERNEL_XSPL", "8"))
            xw = TOK // _XSPL

            def wslice(sl):
                nc.sync.dma_start(
                    wqkv_sb[:, :, ds(sl * w_, w_)],
                    wqkv_d[:, :, ds(sl * w_, w_)].rearrange("k p e -> p k e"),
                )

            def xslice(sl):
                nc.sync.dma_start(
                    xall[:, :, ds(sl * xw, xw)],
                    xT_d[:, :, ds(sl * xw, xw)].rearrange("k p t -> p k t"),
                )

            # interleave so tile 0's deps (w q+k cols, x slice 0) land first
            wslice(0)
            wslice(1)
            xslice(0)
            for sl in range(2, nqk):
                wslice(sl)
            for sl in range(nqk, _WSPL):
                wslice(sl)
            for sl in range(1, _XSPL):
                xslice(sl)
            wo_sb = wp.tile([128, KC, E], bf16)
            _OSPL = int(os.environ.get("KERNEL_OSPL", "2"))
            for sl in range(_OSPL):
                w2 = E // _OSPL
                nc.sync.dma_start(
                    wo_sb[:, :, ds(sl * w2, w2)],
                    wo_d[:, :, ds(sl * w2, w2)].rearrange("k p e -> p k e"),
                )
            ones = wp.tile([1, 128], bf16)
            nc.vector.memset(ones, 1.0)
            ident = wp.tile([128, 128], bf16)
            make_identity(nc, ident)

            def warmup():
                # Spin PE on dummy transposes during the initial DMA wait so
                # the p-state ramp is at full clock when phase1(0) starts.
                nwarm = int(os.environ.get("K_WARM", "100"))
                if nwarm == 0:
                    return
                warm = ps_qkv.tile([128, 128], bf16, tag="warm", bufs=1)
                for _ in range(nwarm):
                    nc.tensor.transpose(warm, ident, ident)

            def fill_pe():
                # Lowest-priority dummy transposes: the list scheduler only
                # places them when no real PE work is ready, keeping the
                # p-state ramp from resetting during PE gaps.
                nfill = int(os.environ.get("K_FILL", "0"))
                if nfill == 0:
                    return
                warm = ps_qkv.tile([128, 128], bf16, tag="warm2", bufs=1)
                saved = tc.cur_priority
                tc.cur_priority = 10**7
                for k in range(nfill):
                    nc.tensor.transpose(warm, ident, ident)
                tc.cur_priority = saved

            def phase1(i):
                # ---- phase 1: qkv = x @ Wqkv + b   (token-major [tok, 3E])
                qkv = qp.tile([128, 3 * E], bf16)
                for j in range(6):  # 6 x 512 output channels
                    ps = ps_qkv.tile([128, 512], f32)
                    for k in range(KC):
                        nc.tensor.matmul(
                            ps,
                            xall[:, k, ts(i, PT)],
                            wqkv_sb[:, k, ds(j * 512, 512)],
                            start=(k == 0),
                            stop=False,
                        )
                    nc.tensor.matmul(
                        ps,
                        ones[0:1, :],
                        bq1[0:1, ds(j * 512, 512)],
                        start=False,
                        stop=True,
                    )
                    nc.scalar.copy(qkv[:, ds(j * 512, 512)], ps)
                return qkv

            HG = H - _HL  # gpsimd-lane head count

            # DVE-lane h-chunks: split products/trees/exp/AV into ranges so
            # the exp->AV bounce pipelines within a tile instead of waiting
            # for the whole lane.
            _VCH = int(os.environ.get("K_VCH", "2"))
            vch = []
            lo = 0
            for c in range(_VCH):
                hi = (_HL * (c + 1)) // _VCH
                vch.append((lo, hi))
                lo = hi

            _ZENG = nc.vector
            _ZACT = os.environ.get("K_ZACT", "0") == "1"
            _TW = int(os.environ.get("K_TW", "1"))  # t-tree stop width
            _NOTT = os.environ.get("K_NOTT", "0") == "1"  # no t-tree: PE
            _TWP = int(os.environ.get("K_TWP", "16"))  # t-cols left to PE
            # transposes accumulate all 16 t-columns straight from pav

            def attn_a(i, qkv):
                """Products, trees, exp, lane-A softmax stats. ACT order here:
                vt first (ready at tile start), then exp_v/exp_g."""
                k_ap = qkv[:, E : 2 * E].rearrange("p (u t d) -> p u t d", t=H, u=1)
                v_ap = qkv[:, 2 * E : 3 * E].rearrange("p (t d) -> p d t", t=H)

                vt = ap_.tile([128, DH, H], bf16)  # v as [d, t]
                nc.scalar.copy(vt, v_ap)

                def sprod(eng, dst, base, lo, hi):
                    q_ap = qkv[:, ds(lo * DH, (hi - lo) * DH)].rearrange(
                        "p (h u d) -> p h u d", h=hi - lo, u=1
                    )
                    eng.tensor_mul(
                        dst[:, ds(lo - base, hi - lo), :, :],
                        q_ap.broadcast_to([128, hi - lo, H, DH]),
                        k_ap.broadcast_to([128, hi - lo, H, DH]),
                    )

                # ---- phase 2a: score products + d-tree (two decoupled lanes)
                psc_v = pp.tile([128, _HL, H, DH], bf16, tag="prodv")
                psc_g = pp.tile([128, HG, H, DH], bf16, tag="prodg")
                sprod(nc.gpsimd, psc_g, _HL, _HL, H)
                ex_v = ap_.tile([128, _HL, H], bf16)
                ex_g = ap_.tile([128, HG, H], bf16)
                zr_v = ap_.tile([128, _HL], f32)
                zb_v = ap_.tile([128, _HL], f32)
                for lo, hi in vch:
                    glo = max(lo, _SP)  # gpsimd-assisted sub-range
                    if glo > lo:
                        sprod(nc.vector, psc_v, 0, lo, glo)
                    if hi > glo:
                        sprod(nc.gpsimd, psc_v, 0, glo, hi)
                    tree(nc.vector, psc_v[:, ds(lo, hi - lo), :, :], DH)
                    # softmax exp (no max-sub; |scores/32| is small)
                    if _ZACT:
                        for h in range(lo, hi):
                            nc.scalar.activation(
                                ex_v[:, h, :],
                                psc_v[:, h, :, 0],
                                AF.Exp,
                                scale=float(E) ** -0.5,
                                accum_out=zr_v[:, h : h + 1],
                            )
                    else:
                        nc.scalar.activation(
                            ex_v[:, ds(lo, hi - lo), :],
                            psc_v[:, ds(lo, hi - lo), :, 0],
                            AF.Exp,
                            scale=float(E) ** -0.5,
                        )
                        _ZENG.reduce_sum(
                            zr_v[:, ds(lo, hi - lo)],
                            ex_v[:, ds(lo, hi - lo), :],
                            axis=mybir.AxisListType.X,
                        )
                tree(nc.gpsimd, psc_g, DH)
                zr_g = ap_.tile([128, HG], f32)
                if _ZACT:
                    for h in range(HG):
                        nc.scalar.activation(
                            ex_g[:, h, :],
                            psc_g[:, h, :, 0],
                            AF.Exp,
                            scale=float(E) ** -0.5,
                            accum_out=zr_g[:, h : h + 1],
                        )
                else:
                    nc.scalar.activation(
                        ex_g, psc_g[:, :, :, 0], AF.Exp, scale=float(E) ** -0.5
                    )
                nc.vector.reciprocal(zb_v, zr_v)
                zb_g = ap_.tile([128, HG], f32)
                if _NOTT:
                    # normalize A in place; extraction is then a plain
                    # accumulate-transpose from pav
                    for lo, hi in vch:
                        nc.vector.tensor_mul(
                            ex_v[:, ds(lo, hi - lo), :],
                            ex_v[:, ds(lo, hi - lo), :],
                            zb_v[:, ds(lo, hi - lo)]
                            .rearrange("p (h u) -> p h u", u=1)
                            .broadcast_to([128, hi - lo, H]),
                        )
                    if not _ZACT:
                        _ZENG.reduce_sum(zr_g, ex_g, axis=mybir.AxisListType.X)
                    nc.vector.reciprocal(zb_g, zr_g)
                    nc.gpsimd.tensor_mul(
                        ex_g,
                        ex_g,
                        zb_g.rearrange("p (h u) -> p h u", u=1).broadcast_to(
                            [128, HG, H]
                        ),
                    )
                return vt, ex_v, ex_g, zb_v, zb_g, zr_g

            def attn_b(i, qkv, vt, ex_v, ex_g, zb_v, zb_g, zr_g):
                # ---- phase 2b: o = exp(S) @ v per token (unnormalized)
                vt_b = vt.rearrange("p (u d) t -> p u d t", u=1)
                pav_v = pp.tile([128, _HL, DH, H], bf16, tag="prodv")
                pav_g = pp.tile([128, HG, DH, H], bf16, tag="prodg")

                def aprod(eng, dst, exl, lo, hi):
                    eng.tensor_mul(
                        dst[:, ds(lo, hi - lo), :, :],
                        exl[:, ds(lo, hi - lo), :]
                        .rearrange("p h (u t) -> p h u t", u=1)
                        .broadcast_to([128, hi - lo, DH, H]),
                        vt_b.broadcast_to([128, hi - lo, DH, H]),
                    )

                aprod(nc.gpsimd, pav_g, ex_g, 0, HG)
                if _AP < _HL:  # off-lane AV assist on gpsimd
                    aprod(nc.gpsimd, pav_v, ex_v, _AP, _HL)
                for lo, hi in vch:
                    if lo < _AP:
                        aprod(nc.vector, pav_v, ex_v, lo, min(hi, _AP))
                    tree(nc.vector, pav_v[:, ds(lo, hi - lo), :, :], H,
                         stop_w=_TWP if _NOTT else _TW)

                if not _NOTT:
                    # gp-lane 1/Z on DVE after its own AV products (exp_g is
                    # long done by then), so DVE never stalls on the gp lane.
                    if not _ZACT:
                        _ZENG.reduce_sum(zr_g, ex_g, axis=mybir.AxisListType.X)
                    nc.vector.reciprocal(zb_g, zr_g)
                    tree(nc.gpsimd, pav_g, H, stop_w=_TW)
                return pav_v, pav_g, zb_v, zb_g

            def tail(i, pav_v, pav_g, zb_v, zb_g):
                """Transpose + o_proj, deferred one tile so the gp lane's
                late finish never blocks the next tile's exp on ACT."""
                pst = ps_t.tile([128, KC, 128], bf16)
                if _NOTT:
                    # transpose straight out of pav, accumulating all 16
                    # t-columns per chunk in PSUM (the AV t-reduction happens
                    # on the PE, not the vector engines)
                    for m in range(KC):
                        h0, h1 = 2 * m, 2 * m + 2
                        srcs = []
                        if h0 < _HL:
                            ve = min(h1, _HL)
                            srcs.append((pav_v, h0, ve, 0))
                        if h1 > _HL:
                            gs = max(h0, _HL)
                            srcs.append((pav_g, gs - _HL, h1 - _HL, (gs - h0) * DH))
                        for pav, a, b, coff in srcs:
                            for w in range(_TWP):
                                nc.tensor.matmul(
                                    pst[coff : coff + (b - a) * DH, m, :],
                                    pav[:, ds(a, b - a), :, w],
                                    ident,
                                    is_transpose=True,
                                    start=(w == 0),
                                    stop=(w == _TWP - 1),
                                )
                else:
                    ocs = []
                    for w in range(_TW):
                        ocs.append(
                            op_.tile([128, E], bf16, tag=f"oc{w}", name=f"oc{w}")
                        )
                    for h in range(H):
                        pav, zb, hh = (
                            (pav_v, zb_v, h) if h < _HL else (pav_g, zb_g, h - _HL)
                        )
                        for w in range(_TW):
                            nc.scalar.activation(
                                ocs[w][:, ds(h * DH, DH)],
                                pav[:, hh, :, w],
                                AF.Copy,
                                scale=zb[:, hh : hh + 1],
                            )
                    for m in range(KC):
                        for w in range(_TW):
                            nc.tensor.matmul(
                                pst[:, m, :],
                                ocs[w][:, ts(m, 128)],
                                ident,
                                is_transpose=True,
                                start=(w == 0),
                                stop=(w == _TW - 1),
                            )
                oT = op_.tile([128, KC, 128], bf16)
                nc.scalar.copy(oT[:, 0:4, :], pst[:, 0:4, :])
                nc.scalar.copy(oT[:, 4:8, :], pst[:, 4:8, :])

                # ---- phase 3: y = o @ Wo + bo
                ysb = yp.tile([128, E], f32)
                for j in range(2):
                    psy = ps_y.tile([128, 512], f32)
                    for m in range(KC):
                        nc.tensor.matmul(
                            psy,
                            oT[:, m, :],
                            wo_sb[:, m, ds(j * 512, 512)],
                            start=(m == 0),
                            stop=False,
                        )
                    nc.tensor.matmul(
                        psy,
                        ones[0:1, :],
                        bo1[0:1, ds(j * 512, 512)],
                        start=False,
                        stop=True,
                    )
                    nc.scalar.copy(ysb[:, ds(j * 512, 512)], psy)
                nc.sync.dma_start(y_d[ts(i, PT), :], ysb)

            if _PIPE:
                warmup()
                fill_pe()
                qkv_next = phase1(0)
                prev = None
                for i in range(NT):
                    qkv_cur = qkv_next
                    a = attn_a(i, qkv_cur)
                    if prev is not None:
                        tail(i - 1, *prev)
                    if i + 1 < NT:
                        qkv_next = phase1(i + 1)
                    prev = attn_b(i, qkv_cur, *a)
                tail(NT - 1, *prev)
            else:
                for i in range(NT):
                    qkv = phase1(i)
                    tail(i, *attn_b(i, qkv, *attn_a(i, qkv)))

    _legalize_waits(nc, mybir)
    return nc


def _legalize_waits(nc, mybir):
    """This walrus build allows only ONE sync wait per engine instruction.
    Split extra waits into standalone same-engine EventSemaphore insts."""
    for f in nc.m.functions:
        for b in f.blocks:
            newl = []
            for inst in b.instructions:
                si = getattr(inst, "sync_info", None)
                ow = list(si.on_wait) if si and si.on_wait else []
                if len(ow) > 1:
                    for w in ow[:-1]:
                        newl.append(
                            mybir.InstEventSemaphore(
                                name=f"WS-{nc.next_id()}",
                                engine=inst.engine,
                                sync_info=mybir.SyncInfo(on_wait=[w], on_update=[]),
                            )
                        )
                    si.on_wait = [ow[-1]]
                newl.append(inst)
            b.instructions = newl


def _prep_weights(w_qkv, b_qkv, w_o, b_o):
    # permute fused-qkv columns: orig e = h*192 + part*64 + d
    #                           new  e = part*1024 + h*64 + d
    part, h, d = np.meshgrid(
        np.arange(3), np.arange(H), np.arange(DH), indexing="ij"
    )
    perm = (h * 192 + part * 64 + d).reshape(-1)
    wq = np.ascontiguousarray(w_qkv[:, perm]).astype(ml_dtypes.bfloat16)
    bq = np.ascontiguousarray(b_qkv[perm]).astype(ml_dtypes.bfloat16)[None, :]
    wo = np.ascontiguousarray(w_o).astype(ml_dtypes.bfloat16)
    return (
        wq.reshape(KC, 128, 3 * E),
        bq,
        wo.reshape(KC, 128, E),
        np.asarray(b_o).astype(ml_dtypes.bfloat16)[None, :],
    )


def kernel(x, w_qkv, b_qkv, w_o, b_o):
    from concourse.bass_utils import run_bass_kernel_spmd

    if "nc" not in _CACHE:
        _CACHE["nc"] = _build_nc()
    nc = _CACHE["nc"]

    wq, bq, wo, bo = _prep_weights(
        np.asarray(w_qkv, np.float32),
        np.asarray(b_qkv, np.float32),
        np.asarray(w_o, np.float32),
        np.asarray(b_o, np.float32),
    )
    x = np.asarray(x, np.float32)
    in_maps = []
    for c in range(NCORES):
        xc = x[2 * c : 2 * c + 2].reshape(TOK, C)
        xT = np.ascontiguousarray(xc.T).astype(ml_dtypes.bfloat16)
        in_maps.append(
            {
                "xT": xT.reshape(KC, 128, TOK),
                "wqkv": wq,
                "bqkv": bq,
                "wo": wo,
                "bo": bo,
            }
        )

    res = run_bass_kernel_spmd(nc, in_maps, core_ids=list(range(NCORES)))
    out = np.empty((B, S, E), np.float32)
    for c in range(NCORES):
        out[2 * c : 2 * c + 2] = res.results[c]["y"].reshape(2, S, E)
    return out
